# revision 23
# baseline (speedup 1.0000x reference)
"""Trainium2 Bass kernel for a causal single-head attention block.

reference:
    K = x @ Wk; Q = x @ Wq; V = x @ Wv          # x [B,T,C], W [C,H]
    scores = (Q @ K^T) * C**-0.5, causal masked
    out = softmax(scores) @ V                    # [B,T,H]

B=512, T=256, C=384, H=64. Pure data parallel over batch across 8 cores
(64 batches per core); the three projection weights are replicated.

Device-side dataflow (per pair of batches):
    x^T tiles [C(part), T] are pre-transposed on the host (x is read only
    once, so the transpose rides along with the mandatory host->device copy).
    a = [Wk|Wk]^T @ x^T  -> psum a [128, T]:  rows 64:128 = K^T
    b = [Wv|Wq]^T @ x^T  -> psum b [128, T]:  rows 0:64 = V^T, 64:128 = Q^T
    (K^T and Q^T both live at partition base 64 because walrus requires
    matmul lhsT/rhs to start at the same SB partition index.)
  per batch:
    V [t, h] via DMA X-bar transpose of V^T (SBUF->SBUF, off the PE)
    scoresT = K @ Q^T    (transposed scores [s, t]; the s>t block of the
           s-tile-1 half is fully masked and never computed)
    expT = exp(scoresT * scale); causal mask on the two diagonal 128x128
           blocks with one strided gpsimd affine_select
    out_unnorm[t, 0:64], denom[t] = expT^T @ [V | ones]   (ones column folds
           the softmax denominator into the PV matmul)
    out[t, h] = out_unnorm[t, h] * (1 / denom[t])  (broadcast tensor_mul)
"""

import os
import sys

for _p in ("/opt/trn_rl_repo", "/root/.axon_site/_ro/trn_rl_repo"):
    if os.path.isdir(_p) and _p not in sys.path:
        sys.path.append(_p)

from contextlib import ExitStack

import ml_dtypes
import numpy as np

import concourse.bass as bass
import concourse.tile as tile
from concourse import bacc, mybir
from concourse.bass_utils import run_bass_kernel_spmd
from concourse.masks import make_identity

B, T, C, H = 512, 256, 384, 64
N_CORES = 8
BPC = B // N_CORES  # batches per core
SCALE = float(C) ** -0.5
NCT = C // 128  # contraction tiles for the projections

F32 = mybir.dt.float32


class Cfg:
    cdt = mybir.dt.bfloat16  # compute dtype on the PE array
    np_cdt = ml_dtypes.bfloat16
    g = 8     # batches per DMA group
    pair = 2  # batches per projection matmul (N = pair*T <= 1024 for bf16)
    chunk = 8  # batches per x-load DMA (multiple of pair)
    split_exp = False  # two exp ops (s0 / s1) instead of one
    pair_scores = True  # pair-level scores psum + one exp per pair
    preset_vones = True  # ones-columns pre-set in fixed rotated slots
    out_bf16 = True  # stage+store out as bf16; host upcasts
    skip_mask = False  # timing experiment only: wrong results when True
    # timing decomposition: "dma" | "projmm" | "proj" | "exp" | "full"
    stage_upto = "full"
    o2sc1 = False  # psum: o_ps bufs=2, sc_ps bufs=1
    bufs_plus = True  # deeper sbuf pools
    block_proj = False  # dead end: psum bank limit caps matmul N at 512
    vones_act = False  # vones copy on ACT instead of DVE
    bufs2 = True  # +2 pool depth instead of +1
    acopy_dve = True  # a-copy on DVE instead of ACT
    v_dma = False  # V transpose via DMA X-bar (slow: ~2us/op serialized)
    a1o2 = True  # psum: a_ps bufs=1, o_ps bufs=2 (v_ps needs the 8th bank)
    skew = 1  # pairs of lag between scores/exp and PV
    mul_ts = False  # normalize via 4x tensor_scalar_mul (1-read) vs tensor_tensor (2-read)
    b_split = False  # b-copy: V^T half on ACT, Q^T half on DVE (balance engines)
    ab_merge = False  # one 2-bank psum->sbuf copy for a+b (fewer DVE instrs)
    host_div = False  # ship out_unnorm + denom; host divides (kills recip+mul)
    mask1 = True  # one 4-block affine_select per pair (expT padded to 512/batch)
    out_sync = True  # out-store DMA dispatch on idle SP SEQ (not ACT SEQ)
    x_alt_q = False  # alternate x-load chunks between sync/scalar HWDGE rings
    out_swdge = False  # out-stores via gpsimd SWDGE (frees HWDGE rings for x)
    loop_r = 0  # if >1, wrap the whole body in a For_i repeat loop (timing)


def build_body(ctx, tc, out, xT, wab, n_b, cfg, dbg=None):
    nc = tc.nc
    cdt = cfg.cdt
    g = cfg.g
    pair = cfg.pair
    n_groups = n_b // g

    consts = ctx.enter_context(tc.tile_pool(name="consts", bufs=1))
    bp = (2 if cfg.bufs2 else 1) if cfg.bufs_plus else 0
    xbufs = (3 + bp) if cfg.chunk <= 8 else 3
    xpool = ctx.enter_context(tc.tile_pool(name="x", bufs=xbufs))
    kqpool = ctx.enter_context(tc.tile_pool(name="kq", bufs=3 + bp))
    epool = ctx.enter_context(tc.tile_pool(name="exp", bufs=3 + bp))
    opool = ctx.enter_context(tc.tile_pool(name="o", bufs=2))
    spool = ctx.enter_context(tc.tile_pool(name="small", bufs=6))
    psum = ctx.enter_context(tc.tile_pool(name="ps", bufs=2, space="PSUM"))

    # --- constants ---------------------------------------------------------
    # wab [C, 4H] = [0|Wk|Wv|Wq]: a-lhsT = cols 0:128, b-lhsT = cols 128:256
    # (zeros so scores can use full-128-partition operands -> FWL weight loads)
    # host pre-arranges wab as [c(128), ct, 4H] so this DMA is contiguous
    wab_sb = consts.tile([128, NCT, 4 * H], cdt)
    nc.sync.dma_start(out=wab_sb, in_=wab)
    ident64 = consts.tile([64, 64], cdt)
    make_identity(nc, ident64)
    vones_slots = []
    for i in range(4):
        # inner extent 80 (16-elem multiple) so each (j, tt) V block is a
        # 16B-aligned contiguous DMA-transpose destination; col H holds the
        # folded-denominator ones column
        vs = consts.tile([128, pair, 2, 80], cdt, name=f"vones{i}", tag=f"vones{i}")
        nc.gpsimd.memset(vs[:, :, :, H : H + 1], 1.0)
        vones_slots.append(vs)

    def body(iv=None):
        n_pairs = n_b // pair
        out_tiles = {}
        xg_tiles = {}

        # pu = batches per projection unit (matmul N = pu*T <= 1024 bf16)
        pu = 2 * pair if cfg.block_proj else pair
        ab_bufs = 1 if cfg.block_proj else 2

        def stage_proj(u):
            # u = projection-unit index (pu batches each)
            ub0 = u * pu  # first batch of the unit (core-local)
            grp = ub0 // g
            if ub0 % g == 0:
                odt = cdt if cfg.out_bf16 else F32
                ow = H + 1 if cfg.host_div else H
                out_tiles[grp] = opool.tile(
                    [128, g, 2, ow], odt, tag="osb", name=f"osb{grp}"
                )
            ch = cfg.chunk  # may exceed g (x chunks are group-independent)
            if ub0 % ch == 0:
                xgc = xpool.tile([128, ch, NCT, T], cdt, tag="xg")
                xg_tiles[ub0 // ch] = xgc
                # xT host layout is [c(128), b, ct, t]: contiguous runs of
                # ch*NCT*T elems per partition (12KB) -> full DMA throughput
                xq = nc.scalar if (cfg.x_alt_q and (ub0 // ch) % 2) else nc.sync
                xq.dma_start(out=xgc, in_=xT[:, ub0 : ub0 + ch])
            xgc = xg_tiles[ub0 // ch]
            xoff = ub0 % ch
            if cfg.stage_upto == "dma":
                return None, None
            # a: rows 64:128 = K^T ; b: rows 0:64 = V^T, 64:128 = Q^T
            if cfg.ab_merge:
                ab_ps = psum.tile([128, 2, pu, T], F32, tag="abps", bufs=2)
                a_ps = ab_ps[:, 0]
                b_ps = ab_ps[:, 1]
            else:
                a_ps = psum.tile(
                    [128, pu, T], F32, tag="aps", bufs=1 if cfg.a1o2 else ab_bufs
                )
                b_ps = psum.tile([128, pu, T], F32, tag="bps", bufs=ab_bufs)
            for ct in range(NCT):
                nc.tensor.matmul(
                    a_ps,
                    wab_sb[:, ct, 0:128],
                    xgc[:, xoff : xoff + pu, ct, :],
                    start=(ct == 0),
                    stop=(ct == NCT - 1),
                )
            for ct in range(NCT):
                nc.tensor.matmul(
                    b_ps,
                    wab_sb[:, ct, 128:256],
                    xgc[:, xoff : xoff + pu, ct, :],
                    start=(ct == 0),
                    stop=(ct == NCT - 1),
                )
            if cfg.stage_upto == "projmm":
                return None, None
            if cfg.ab_merge:
                ab_sb = kqpool.tile([128, 2, pu, T], cdt, tag="absb")
                nc.vector.tensor_copy(ab_sb, ab_ps)
                return ab_sb[:, 0], ab_sb[:, 1]
            a_sb = kqpool.tile([128, pu, T], cdt, tag="asb")
            if cfg.acopy_dve:
                nc.vector.tensor_copy(a_sb[64:128], a_ps[64:128])
            else:
                nc.scalar.copy(a_sb[64:128], a_ps[64:128])
            b_sb = kqpool.tile([128, pu, T], cdt, tag="bsb")
            if cfg.b_split:
                nc.scalar.copy(b_sb[0:64], b_ps[0:64])
                nc.vector.tensor_copy(b_sb[64:128], b_ps[64:128])
            else:
                nc.vector.tensor_copy(b_sb, b_ps)
            return a_sb, b_sb

        def stage_scores(p, a_sb, b_sb):
            """Transposes + scores + exp + mask for pair p; returns the
            state stage_pv needs (runs cfg.skew pairs later)."""
            grp, pr = divmod(p, g // pair)
            out_sb = out_tiles[grp]
            b0 = pr * pair
            jj0 = (p * pair) % pu  # sub-pair offset inside the proj unit
            if cfg.stage_upto in ("dma", "projmm"):
                # keep the out staging + DMA so only MMs/copies differ
                nc.vector.tensor_copy(
                    out_sb[:, b0 : b0 + pair, :, 0:8],
                    vones_slots[0][:, :, :, 0:8],
                )
                odma = nc.sync if cfg.out_sync else (nc.gpsimd if cfg.out_swdge else nc.scalar)
                odma.dma_start(
                    out=out[:, grp * g + b0 : grp * g + b0 + pair],
                    in_=out_sb[:, b0 : b0 + pair, :, :],
                )
                return None
            if cfg.stage_upto == "proj":
                # timing diagnostic: fake the output from b_sb, skip attention
                nc.vector.tensor_copy(
                    out_sb[:, b0 : b0 + pair, :, :],
                    b_sb.rearrange("p b (tt h2) -> p b tt h2", tt=2)[:, jj0 : jj0 + pair, :, 0:H],
                )
                odma = nc.sync if cfg.out_sync else (nc.gpsimd if cfg.out_swdge else nc.scalar)
                odma.dma_start(
                    out=out[:, grp * g + b0 : grp * g + b0 + pair],
                    in_=out_sb[:, b0 : b0 + pair, :, :],
                )
                return None
            vones_p = vones_slots[p % 4]
            if cfg.v_dma:
                # V natural [t, h] via DMA X-bar transpose (dead end: ~2us
                # per serialized SB->SB transpose op)
                for j in range(pair):
                    for tt in range(2):
                        nc.sync.dma_start(
                            out=vones_p[:, j, tt, 0:H],
                            in_=b_sb[0:64, jj0 + j, tt * 128 : (tt + 1) * 128],
                            transpose=True,
                        )
            else:
                # V natural [t, h] for the pair via PE transpose of V^T
                v_ps = psum.tile([128, pair, 2, H], cdt, tag="vps", bufs=1)
                for j in range(pair):
                    for tt in range(2):
                        nc.tensor.transpose(
                            v_ps[:, j, tt, :],
                            b_sb[0:64, jj0 + j, tt * 128 : (tt + 1) * 128],
                            ident64,
                        )
                if cfg.vones_act:
                    nc.scalar.copy(vones_p[:, :, :, 0:H], v_ps)
                else:
                    nc.vector.tensor_copy(vones_p[:, :, :, 0:H], v_ps)
            if cfg.pair_scores:
                # one [128, pair, 384] psum (2 banks), one exp, one mask
                sc_pp = psum.tile([128, pair, 512], F32, tag="scps", bufs=1)
                for j in range(pair):
                    nc.tensor.matmul(
                        sc_pp[:, j, 0:T],
                        a_sb[64:128, jj0 + j, 0:128],
                        b_sb[64:128, jj0 + j, :],
                        start=True,
                        stop=True,
                    )
                    nc.tensor.matmul(
                        sc_pp[:, j, T : T + 128],
                        a_sb[64:128, jj0 + j, 128:T],
                        b_sb[64:128, jj0 + j, 128:T],
                        start=True,
                        stop=True,
                    )
                ew = 512 if cfg.mask1 else T + 128
                expT_p = epool.tile([128, pair, ew], cdt, tag="expT")
                nc.scalar.activation(
                    expT_p[:, :, 0 : T + 128],
                    sc_pp[:, :, 0 : T + 128],
                    mybir.ActivationFunctionType.Exp,
                    scale=SCALE,
                )
                if cfg.skip_mask:
                    pass
                elif cfg.mask1:
                    # 512-padded per-batch extent puts the 4 diagonal blocks
                    # at a uniform 256-elem stride -> one 3D-AP affine_select
                    # (walrus rejects the 4D unpadded form)
                    blocks = expT_p.rearrange("p b t -> p (b t)").rearrange(
                        "p (k c) -> p k c", c=128
                    )[:, 0::2, :]
                    nc.gpsimd.affine_select(
                        out=blocks,
                        in_=blocks,
                        compare_op=mybir.AluOpType.is_ge,
                        fill=0.0,
                        base=0,
                        pattern=[[0, 4], [1, 128]],
                        channel_multiplier=-1,
                    )
                else:
                    for j in range(pair):
                        blocks = expT_p[:, j, 0 : T + 128].rearrange(
                            "p (n c) -> p n c", c=128
                        )[:, 0::2, :]
                        nc.gpsimd.affine_select(
                            out=blocks,
                            in_=blocks,
                            compare_op=mybir.AluOpType.is_ge,
                            fill=0.0,
                            base=0,
                            pattern=[[0, 2], [1, 128]],
                            channel_multiplier=-1,
                        )
            expTs = []
            for j in range(pair):
                bb = b0 + j

                if cfg.pair_scores:
                    expT = expT_p[:, j, 0 : T + 128]
                else:
                    # transposed scores, one psum bank [128, 384]:
                    # cols 0:T = s-tile 0 (all t), T:T+128 = s-tile 1 (t>=128)
                    sc_ps = psum.tile(
                        [128, T + 128], F32, tag="scps", bufs=1 if cfg.o2sc1 else 2
                    )
                    nc.tensor.matmul(
                        sc_ps[:, 0:T],
                        a_sb[64:128, jj0 + j, 0:128],
                        b_sb[64:128, jj0 + j, :],
                        start=True,
                        stop=True,
                    )
                    nc.tensor.matmul(
                        sc_ps[:, T : T + 128],
                        a_sb[64:128, jj0 + j, 128:T],
                        b_sb[64:128, jj0 + j, 128:T],
                        start=True,
                        stop=True,
                    )

                    expT = epool.tile([128, T + 128], cdt, tag="expT")
                    if cfg.split_exp:
                        nc.scalar.activation(
                            expT[:, 0:T],
                            sc_ps[:, 0:T],
                            mybir.ActivationFunctionType.Exp,
                            scale=SCALE,
                        )
                        nc.scalar.activation(
                            expT[:, T : T + 128],
                            sc_ps[:, T : T + 128],
                            mybir.ActivationFunctionType.Exp,
                            scale=SCALE,
                        )
                    else:
                        nc.scalar.activation(
                            expT,
                            sc_ps,
                            mybir.ActivationFunctionType.Exp,
                            scale=SCALE,
                        )
                    # causal mask on both diagonal blocks (cols 0:128 and
                    # 256:384) in one strided op: keep where -s + t >= 0
                    blocks = expT.rearrange("p (n c) -> p n c", c=128)[:, 0::2, :]
                    if not cfg.skip_mask:
                        nc.gpsimd.affine_select(
                            out=blocks,
                            in_=blocks,
                            compare_op=mybir.AluOpType.is_ge,
                            fill=0.0,
                            base=0,
                            pattern=[[0, 2], [1, 128]],
                            channel_multiplier=-1,
                        )

                if cfg.stage_upto == "exp":
                    nc.vector.tensor_copy(
                        out_sb[:, bb, :, :],
                        expT.rearrange("p (n h2) -> p n h2", h2=H)[:, 0:2, :],
                    )
                    continue
                expTs.append(expT)
                if dbg is not None and p == 0 and j == 0:
                    nc.vector.tensor_copy(dbg["kq"][0:64], a_sb[64:128, jj0, :])
                    nc.vector.tensor_copy(dbg["kq"][64:128], b_sb[64:128, jj0, :])
                    nc.vector.tensor_copy(
                        dbg["vones"], vones_p[:, 0, :, 0 : H + 1]
                    )
                    nc.vector.tensor_copy(dbg["expT"], expT)

            if cfg.stage_upto == "exp":
                odma = nc.sync if cfg.out_sync else (nc.gpsimd if cfg.out_swdge else nc.scalar)
                odma.dma_start(
                    out=out[:, grp * g + b0 : grp * g + b0 + pair],
                    in_=out_sb[:, b0 : b0 + pair, :, :],
                )
                return None
            return (p, expTs, vones_p, out_sb, b0, grp)

        def stage_pv(state):
            """PV + normalize + store for pair p (expT is long since ready)."""
            if state is None:
                return
            p, expTs, vones_p, out_sb, b0, grp = state
            o_ps = psum.tile(
                [128, pair, 2, H + 1], F32, tag="ops", bufs=2 if cfg.a1o2 else 1
            )
            for j in range(pair):
                vones = vones_p[:, j, :, 0 : H + 1]
                expT = expTs[j]
                # PV + folded denominator: out_unnorm = expT^T @ [V | 1]
                nc.tensor.matmul(
                    o_ps[:, j, 0, :],
                    expT[:, 0:128],
                    vones[:, 0, :],
                    start=True,
                    stop=True,
                )
                nc.tensor.matmul(
                    o_ps[:, j, 1, :],
                    expT[:, 128:T],
                    vones[:, 0, :],
                    start=True,
                    stop=False,
                )
                nc.tensor.matmul(
                    o_ps[:, j, 1, :],
                    expT[:, T : T + 128],
                    vones[:, 1, :],
                    start=False,
                    stop=True,
                )
            if cfg.host_div:
                nc.vector.tensor_copy(
                    out_sb[:, b0 : b0 + pair, :, :], o_ps[:, :, :, 0 : H + 1]
                )
                odma = nc.sync if cfg.out_sync else (nc.gpsimd if cfg.out_swdge else nc.scalar)
                odma.dma_start(
                    out=out[:, grp * g + b0 : grp * g + b0 + pair],
                    in_=out_sb[:, b0 : b0 + pair, :, :],
                )
                return
            recip = spool.tile([128, pair, 2, 1], F32)
            nc.vector.reciprocal(recip, o_ps[:, :, :, H : H + 1])
            if cfg.mul_ts:
                for j in range(pair):
                    for tt in range(2):
                        nc.vector.tensor_scalar_mul(
                            out_sb[:, b0 + j, tt, :],
                            o_ps[:, j, tt, 0:H],
                            recip[:, j, tt, :],
                        )
            else:
                rbc = bass.AP(
                    tensor=recip.tensor,
                    offset=recip.offset,
                    ap=[recip.ap[0], recip.ap[1], recip.ap[2], [0, H]],
                )
                nc.vector.tensor_mul(
                    out_sb[:, b0 : b0 + pair, :, :], o_ps[:, :, :, 0:H], rbc
                )
            odma = nc.sync if cfg.out_sync else (nc.gpsimd if cfg.out_swdge else nc.scalar)
            odma.dma_start(
                out=out[:, grp * g + b0 : grp * g + b0 + pair],
                in_=out_sb[:, b0 : b0 + pair, :, :],
            )

        # software pipeline: proj one unit ahead of scores; PV cfg.skew
        # pairs behind scores so the PE never waits on exp/mask
        ppu = pu // pair  # pairs per projection unit
        n_units = n_pairs // ppu
        pend = None
        states = []
        for u in range(n_units):
            ab = stage_proj(u)
            if pend is not None:
                for q in range(ppu):
                    states.append(stage_scores((u - 1) * ppu + q, *pend))
                while len(states) > cfg.skew:
                    stage_pv(states.pop(0))
            pend = ab
        for q in range(ppu):
            states.append(stage_scores((n_units - 1) * ppu + q, *pend))
        while states:
            stage_pv(states.pop(0))

    if cfg.loop_r and cfg.loop_r > 1:
        with tc.For_i(0, cfg.loop_r, 1) as iv:
            body(iv)
    else:
        body()


def build_kernel(n_b=BPC, cfg=None, debug_taps=False):
    cfg = cfg or Cfg()
    nc = bacc.Bacc("TRN2", target_bir_lowering=False, debug=False)
    # xT: [c(128), b, ct, t] — the exact SBUF layout, so x-loads are
    # fully contiguous per partition. out: [p, b, tt, h] SBUF-native;
    # host un-permutes (t = tt*128 + p).
    xT = nc.dram_tensor("xT", [128, n_b, NCT, T], cfg.cdt, kind="ExternalInput").ap()
    wab = nc.dram_tensor("wab", [128, NCT, 4 * H], cfg.cdt, kind="ExternalInput").ap()
    odt = cfg.cdt if cfg.out_bf16 else F32
    ow = H + 1 if cfg.host_div else H
    out = nc.dram_tensor("out", [128, n_b, 2, ow], odt, kind="ExternalOutput").ap()
    dbg = None
    dbg_specs = {
        "kq": [128, T],
        "vones": [128, 2, H + 1],
        "expT": [128, T + 128],
    }
    dbg_dram = {}
    if debug_taps:
        dbg_dram = {
            k: nc.dram_tensor(f"dbg_{k}", s, cfg.cdt, kind="ExternalOutput").ap()
            for k, s in dbg_specs.items()
        }

    with tile.TileContext(nc) as tc, ExitStack() as ctx:
        if debug_taps:
            dbgpool = ctx.enter_context(tc.tile_pool(name="dbg", bufs=1))
            dbg = {
                k: dbgpool.tile(s, cfg.cdt, name=f"dbgsb_{k}")
                for k, s in dbg_specs.items()
            }
        build_body(ctx, tc, out, xT, wab, n_b, cfg, dbg=dbg)
        if debug_taps:
            for k in dbg_dram:
                nc.sync.dma_start(out=dbg_dram[k], in_=dbg[k])
    nc.compile()
    return nc


def prep_inputs(x, Wk, Wq, Wv, n_cores=N_CORES, cfg=None):
    """Shard over batch + host-side pre-transpose/cast of x.

    xT shard layout [c(128), b, ct, t]: x[b, t, ct*128+c] -> xT[c, b, ct, t].
    wab layout [c(128), ct, 4H]: wab_full[ct*128+c, m] -> wab[c, ct, m].
    """
    cfg = cfg or Cfg()
    x = np.asarray(x, dtype=np.float32)
    Wk = np.asarray(Wk, dtype=np.float32)
    Wq = np.asarray(Wq, dtype=np.float32)
    Wv = np.asarray(Wv, dtype=np.float32)
    bpc = x.shape[0] // n_cores
    wab_full = np.concatenate([np.zeros_like(Wk), Wk, Wv, Wq], axis=1)
    wab = np.ascontiguousarray(
        wab_full.reshape(NCT, 128, 4 * H).transpose(1, 0, 2)
    ).astype(cfg.np_cdt)
    in_maps = []
    for i in range(n_cores):
        shard = x[i * bpc : (i + 1) * bpc]  # [b, t, C]
        # [b, t, ct, c] -> [c, b, ct, t]
        xTs = np.ascontiguousarray(
            shard.reshape(bpc, T, NCT, 128).transpose(3, 0, 2, 1)
        ).astype(cfg.np_cdt)
        in_maps.append({"xT": xTs, "wab": wab})
    return in_maps


_NC_CACHE = {}


def kernel(x, Wk, Wq, Wv):
    cfg = Cfg()
    key = (
        x.shape[0] // N_CORES, cfg.cdt, cfg.g, cfg.pair, cfg.chunk,
        cfg.out_bf16, cfg.host_div, cfg.ab_merge,
    )
    if key not in _NC_CACHE:
        _NC_CACHE[key] = build_kernel(n_b=key[0], cfg=cfg)
    nc = _NC_CACHE[key]
    in_maps = prep_inputs(x, Wk, Wq, Wv, cfg=cfg)
    res = run_bass_kernel_spmd(nc, in_maps, list(range(N_CORES)))
    # device out is [p, b, tt, h(+denom)] with t = tt*128 + p -> [b, t, h]
    def unshard(arr):
        a = arr.astype(np.float32).transpose(1, 2, 0, 3).reshape(-1, T, arr.shape[-1])
        if cfg.host_div:
            a = a[:, :, 0:H] / a[:, :, H : H + 1]
        return a

    full = np.concatenate([unshard(r["out"]) for r in res.results], axis=0)
    return np.ascontiguousarray(full.astype(np.float32))

